# revision 16
# baseline (speedup 1.0000x reference)
"""Replicated (collective-free) Trainium2 kernel for the dense-graph GNN layer.

Math: with xn = x/||x|| (rows), G = xn@xn.T, d = rsqrt(G@1),
out = (diag(d) G diag(d) x) W.  The N x N Gram matrix is never formed, and
the normalized matrix xn is never materialized (norms cancel):
  t = colsum(xn)  (PE: invn-weighted colsum of raw x)
  u = x @ t,  s = u * invn,  d = rsqrt(s),  f = d * invn
  z = x.T @ diag(f) @ x  (symmetric);  out = diag(f) @ x @ (z @ W)

Distribution: NO collectives.  Each core streams the FULL x from HBM
(DMA-overlapped) and redundantly computes the tiny globals; core r gets a
row-rotated x so a rank-agnostic program emits rows r*1024..(r+1)*1024.

Engine split (measured rates): ACT casts most of the stream + some row
reductions via activation accumulate; DVE does batched bf16 multiplies
(2x mode) + batched reduces + per-tile f-scaling; Pool casts some chunks
and does some per-tile s-dot multiplies; PE runs the colsum during the
stream and the dense z accumulation in phase B.
"""

import os
import sys

import numpy as np

for _p in ("/opt/trn_rl_repo", "/root/.axon_site/_ro/trn_rl_repo"):
    if os.path.isdir(_p) and _p not in sys.path:
        sys.path.insert(0, _p)

import concourse.bacc as bacc
import concourse.mybir as mybir
import concourse.tile as tile
import concourse.masks as masks
from concourse import bass_utils
from concourse.bass_types import AP as _AP

R = 8                 # cores
N, D = 8192, 256
NL = N // R           # 1024 output rows per core
P = 128
NT = N // P           # 64 row tiles streamed per core
CH = 8                # tiles per input DMA chunk
NCH = NT // CH        # 8 chunks
LT = NL // P          # 8 local (output) tiles = tiles 0..7 of the rotated view
GRP = 8               # tiles per s/g/z pipeline group
NG = NT // GRP
F32 = mybir.dt.float32
BF16 = mybir.dt.bfloat16
AF = mybir.ActivationFunctionType
ALU = mybir.AluOpType

POOL_CAST_CHUNKS = set()        # pool casts measured too slow (3.5 ns/elem)
ACT_SS_CHUNKS = {0}             # chunks whose row-norm accumulate runs on ACT
DVE_SRED_GROUPS = {0, 2, 3, 5, 6, 7}  # s-reduce on DVE (rest per-tile ACT)
POOL_SMUL_GROUPS = {1, 3, 5, 7}  # groups whose s-multiply runs on Pool
ACT_G_GROUPS = {2, 6}           # groups whose f-scale runs per-tile on ACT

_cache = {}


def _rep(ap, n, pos):
    """Insert a stride-0 broadcast dim of size n at free position pos."""
    dims = list(ap.ap)
    dims.insert(pos, [0, n])
    return _AP(ap.tensor, ap.offset, dims)


def _program(tc, x, W, out):
    nc = tc.nc
    with (
        tc.tile_pool(name="persist", bufs=1) as pp,
        tc.tile_pool(name="work", bufs=3) as wp,
        tc.tile_pool(name="psacc", bufs=1, space="PSUM") as psp,
        tc.tile_pool(name="pswork", bufs=2, space="PSUM") as psw,
    ):
        xball = pp.tile([P, NT * D], BF16)    # bf16 x, tile i at [:, i*D:]
        g_all = pp.tile([P, NT * D], BF16)    # f * x
        xbT = pp.tile([P, 2 * NL], BF16)      # local x.T; chunk h at h*NL+i*P
        ss = pp.tile([P, NT], F32)            # ||x||^2
        rss = pp.tile([P, NT], F32)           # 1/||x||^2
        invnb = pp.tile([P, NT], BF16)        # 1/||x||
        u_all = pp.tile([P, NT], F32)         # x @ t
        s_all = pp.tile([P, NT], F32)         # degrees
        rec = pp.tile([P, NT], F32)           # 1/s
        d_all = pp.tile([P, NT], F32)         # rsqrt(s)
        f_all = pp.tile([P, NT], F32)         # d / ||x||
        t16 = pp.tile([1, D], BF16)
        tb16 = pp.tile([P, D], BF16)
        W_sb = pp.tile([P, 2 * D], F32)
        Wb = pp.tile([P, 2 * D], BF16)
        zb = pp.tile([P, 2 * D], BF16)
        zwb = pp.tile([P, 2 * D], BF16)
        ones1 = pp.tile([1, P], BF16)
        identb = pp.tile([P, P], BF16)

        nc.gpsimd.memset(ones1[:], 1.0)
        masks.make_identity(nc, identb[:])

        psum_t = psp.tile([1, D], F32, padded_shape=[1, 512])
        pz0 = psp.tile([P, D], F32, padded_shape=[P, 512])
        pz1 = psp.tile([P, D], F32, padded_shape=[P, 512])

        # ---- phase A: stream x; cast, row norms, colsum t, local x.T ----
        for c in range(NCH):
            cs = slice(c * CH, (c + 1) * CH)
            xs = wp.tile([P, CH * D], F32, tag="xs", name=f"xs{c}")
            src = _AP(x.tensor, x.offset + c * CH * P * D,
                      [[D, P], [P * D, CH], [1, D]])
            nc.sync.dma_start(xs[:].rearrange("p (j d) -> p j d", j=CH), src)
            xbc = xball[:, c * CH * D:(c + 1) * CH * D]
            if c in POOL_CAST_CHUNKS:
                nc.gpsimd.tensor_copy(xbc, xs[:])
            else:
                nc.scalar.copy(xbc, xs[:])
            if c in ACT_SS_CHUNKS:
                # row sum-squares via ACT Square+accumulate (from fp32 xs)
                for j in range(CH):
                    i = c * CH + j
                    sq = wp.tile([P, D], BF16, tag="sq", name=f"sq{i}")
                    nc.scalar.activation(sq[:], xs[:, j * D:(j + 1) * D],
                                         AF.Square, accum_out=ss[:, i:i + 1])
            else:
                scrq = wp.tile([P, CH * D], BF16, tag="scrq", name=f"scrq{c}")
                nc.vector.tensor_mul(
                    scrq[:].rearrange("p (j d) -> p j d", j=CH),
                    xbc.rearrange("p (j d) -> p j d", j=CH),
                    xbc.rearrange("p (j d) -> p j d", j=CH))
                with nc.allow_low_precision("row-norm accumulate"):
                    nc.vector.tensor_reduce(
                        ss[:, cs], scrq[:].rearrange("p (j d) -> p j d", j=CH),
                        axis=mybir.AxisListType.X, op=ALU.add)
            nc.vector.reciprocal(rss[:, cs], ss[:, cs])
            nc.scalar.activation(invnb[:, cs], rss[:, cs], AF.Sqrt)
            for j in range(CH):
                i = c * CH + j
                nc.tensor.matmul(psum_t[:], lhsT=invnb[:, i:i + 1],
                                 rhs=xball[:, i * D:(i + 1) * D],
                                 start=(i == 0), stop=(i == NT - 1))
            if c == 0:  # local tiles: build x.T via PE transposes
                for j in range(CH):
                    i = j
                    for h in range(2):
                        pt = psw.tile([P, P], BF16, tag="pt", name=f"pt{i}_{h}")
                        nc.tensor.transpose(
                            pt[:], xball[:, i * D + h * P: i * D + (h + 1) * P],
                            identb[:])
                        if (i + h) % 2 == 0:
                            nc.vector.tensor_copy(
                                xbT[:, h * NL + i * P: h * NL + (i + 1) * P], pt[:])
                        else:
                            nc.scalar.copy(
                                xbT[:, h * NL + i * P: h * NL + (i + 1) * P], pt[:])

        # W load on the scalar-engine DMA queue (off the x stream)
        for kc in range(2):
            nc.scalar.dma_start(W_sb[:, kc * D:(kc + 1) * D],
                                W[kc * P:(kc + 1) * P, :])
        nc.vector.tensor_copy(Wb[:], W_sb[:])

        # ---- t -> broadcast to all partitions ----
        nc.vector.tensor_copy(t16[:], psum_t[:])
        ptb = psw.tile([P, D], F32, tag="pw", name="ptb")
        nc.tensor.matmul(ptb[:], lhsT=ones1[:], rhs=t16[:], start=True, stop=True)
        nc.vector.tensor_copy(tb16[:], ptb[:])

        # ---- phase B1: degrees and f for every group (no PE work) ----
        for gi in range(NG):
            gs = slice(gi * GRP, (gi + 1) * GRP)
            scrs = wp.tile([P, GRP * D], BF16, tag="scrs", name=f"scrs{gi}")
            if gi in POOL_SMUL_GROUPS:
                for j in range(GRP):
                    i = gi * GRP + j
                    nc.gpsimd.tensor_tensor(
                        scrs[:, j * D:(j + 1) * D],
                        xball[:, i * D:(i + 1) * D], tb16[:], op=ALU.mult)
            else:
                nc.vector.tensor_mul(
                    scrs[:].rearrange("p (t d) -> p t d", t=GRP),
                    xball[:, gi * GRP * D:(gi + 1) * GRP * D].rearrange(
                        "p (t d) -> p t d", t=GRP),
                    _rep(tb16[:], GRP, 1))
            if gi in DVE_SRED_GROUPS:
                with nc.allow_low_precision("degree accumulate"):
                    nc.vector.tensor_reduce(
                        u_all[:, gs],
                        scrs[:].rearrange("p (t d) -> p t d", t=GRP),
                        axis=mybir.AxisListType.X, op=ALU.add)
            else:
                for j in range(GRP):
                    i = gi * GRP + j
                    sq = wp.tile([P, D], BF16, tag="sq", name=f"sr{i}")
                    nc.scalar.activation(sq[:], scrs[:, j * D:(j + 1) * D],
                                         AF.Copy, accum_out=u_all[:, i:i + 1])
            # f = d * invn = sqrt(invn / u)
            nc.vector.reciprocal(rec[:, gs], u_all[:, gs])
            nc.vector.tensor_mul(s_all[:, gs], rec[:, gs], invnb[:, gs])
            nc.scalar.activation(f_all[:, gs], s_all[:, gs], AF.Sqrt)

        # ---- phase B2: g = f*x and one dense z matmul train ----
        for gi in range(NG):
            gs = slice(gi * GRP, (gi + 1) * GRP)
            if gi in ACT_G_GROUPS:
                for j in range(GRP):
                    i = gi * GRP + j
                    nc.scalar.mul(g_all[:, i * D:(i + 1) * D],
                                  xball[:, i * D:(i + 1) * D], f_all[:, i:i + 1])
            else:
                nc.vector.tensor_mul(
                    g_all[:, gi * GRP * D:(gi + 1) * GRP * D].rearrange(
                        "p (t d) -> p t d", t=GRP),
                    xball[:, gi * GRP * D:(gi + 1) * GRP * D].rearrange(
                        "p (t d) -> p t d", t=GRP),
                    _rep(f_all[:, gs], D, 2))
        for gi in range(NG):
            for j in range(GRP):
                i = gi * GRP + j
                for h, pz in ((0, pz0), (1, pz1)):
                    nc.tensor.matmul(
                        pz[:],
                        lhsT=g_all[:, i * D + h * P: i * D + (h + 1) * P],
                        rhs=xball[:, i * D:(i + 1) * D],
                        start=(i == 0), stop=(i == NT - 1))

        nc.vector.tensor_copy(zb[:, 0:D], pz0[:])
        nc.vector.tensor_copy(zb[:, D:2 * D], pz1[:])

        # ---- zw = z @ W ----
        for m in range(2):
            pzw = psw.tile([P, D], F32, tag="pw", name=f"pzw{m}")
            for h in range(2):
                nc.tensor.matmul(pzw[:],
                                 lhsT=zb[:, h * D + m * P: h * D + (m + 1) * P],
                                 rhs=Wb[:, h * D:(h + 1) * D],
                                 start=(h == 0), stop=(h == 1))
            nc.vector.tensor_copy(zwb[:, m * D:(m + 1) * D], pzw[:])

        # ---- phase C: out = diag(f) x_local (zw) ----
        for i in range(LT):
            po = psw.tile([P, D], F32, tag="pw", name=f"po{i}")
            for h in range(2):
                nc.tensor.matmul(po[:],
                                 lhsT=xbT[:, h * NL + i * P: h * NL + (i + 1) * P],
                                 rhs=zwb[:, h * D:(h + 1) * D],
                                 start=(h == 0), stop=(h == 1))
            o_sb = wp.tile([P, D], F32, tag="o", name=f"o{i}")
            nc.vector.tensor_scalar_mul(o_sb[:], po[:], f_all[:, i:i + 1])
            nc.sync.dma_start(out[i * P:(i + 1) * P, :], o_sb[:])


def _build():
    nc = bacc.Bacc("TRN2", target_bir_lowering=False, debug=False, num_devices=R)
    x = nc.dram_tensor("x", [N, D], F32, kind="ExternalInput")
    W = nc.dram_tensor("W", [D, D], F32, kind="ExternalInput")
    out = nc.dram_tensor("out", [NL, D], F32, kind="ExternalOutput")
    with tile.TileContext(nc) as tc:
        _program(tc, x.ap() if hasattr(x, "ap") else x,
                 W.ap() if hasattr(W, "ap") else W,
                 out.ap() if hasattr(out, "ap") else out)
    nc.finalize()
    return nc


def _run(inputs, trace=False):
    if "nc" not in _cache:
        _cache["nc"] = _build()
    nc = _cache["nc"]
    x = np.ascontiguousarray(inputs["x"], dtype=np.float32)
    W = np.ascontiguousarray(inputs["W"], dtype=np.float32)
    in_maps = []
    for r in range(R):
        xr = np.concatenate([x[r * NL:], x[:r * NL]], axis=0) if r else x
        in_maps.append({"x": xr, "W": W})
    res = bass_utils.run_bass_kernel_spmd(
        nc, in_maps, core_ids=list(range(R)), trace=trace,
    )
    out = np.concatenate([res.results[r]["out"] for r in range(R)], axis=0)
    return out, res


def kernel(**inputs) -> np.ndarray:
    out, _ = _run(inputs, trace=False)
    return out


# revision 17
# speedup vs baseline: 1.0250x; 1.0250x over previous
"""Replicated (collective-free) Trainium2 kernel for the dense-graph GNN layer.

Math: with xn = x/||x|| (rows), G = xn@xn.T, d = rsqrt(G@1),
out = (diag(d) G diag(d) x) W.  The N x N Gram matrix is never formed, and
the normalized matrix xn is never materialized (norms cancel):
  t = colsum(xn)  (PE: invn-weighted colsum of raw x)
  u = x @ t,  s = u * invn,  d = rsqrt(s),  f = d * invn
  z = x.T @ diag(f) @ x  (symmetric);  out = diag(f) @ x @ (z @ W)

Distribution: NO collectives.  Each core streams the FULL x from HBM
(DMA-overlapped) and redundantly computes the tiny globals; core r gets a
row-rotated x so a rank-agnostic program emits rows r*1024..(r+1)*1024.

Engine split (measured rates): ACT casts most of the stream + some row
reductions via activation accumulate; DVE does batched bf16 multiplies
(2x mode) + batched reduces + per-tile f-scaling; Pool casts some chunks
and does some per-tile s-dot multiplies; PE runs the colsum during the
stream and the dense z accumulation in phase B.
"""

import os
import sys

import numpy as np

for _p in ("/opt/trn_rl_repo", "/root/.axon_site/_ro/trn_rl_repo"):
    if os.path.isdir(_p) and _p not in sys.path:
        sys.path.insert(0, _p)

import concourse.bacc as bacc
import concourse.mybir as mybir
import concourse.tile as tile
import concourse.masks as masks
from concourse import bass_utils
from concourse.bass_types import AP as _AP

R = 8                 # cores
N, D = 8192, 256
NL = N // R           # 1024 output rows per core
P = 128
NT = N // P           # 64 row tiles streamed per core
CH = 8                # tiles per input DMA chunk
NCH = NT // CH        # 8 chunks
LT = NL // P          # 8 local (output) tiles = tiles 0..7 of the rotated view
GRP = 8               # tiles per s/g/z pipeline group
NG = NT // GRP
F32 = mybir.dt.float32
BF16 = mybir.dt.bfloat16
AF = mybir.ActivationFunctionType
ALU = mybir.AluOpType

POOL_CAST_CHUNKS = set()        # pool casts measured too slow (3.5 ns/elem)
ACT_SS_CHUNKS = {0}             # chunks whose row-norm accumulate runs on ACT
DVE_SRED_GROUPS = {0, 2, 3, 5, 6, 7}  # s-reduce on DVE (rest ACT)
POOL_SMUL_GROUPS = {1, 3, 5, 7}  # groups whose s-multiply runs on Pool
ACT_G_GROUPS = {2, 6}           # groups whose f-scale runs per-tile on ACT

_cache = {}


def _rep(ap, n, pos):
    """Insert a stride-0 broadcast dim of size n at free position pos."""
    dims = list(ap.ap)
    dims.insert(pos, [0, n])
    return _AP(ap.tensor, ap.offset, dims)


def _program(tc, x, W, out):
    nc = tc.nc
    with (
        tc.tile_pool(name="persist", bufs=1) as pp,
        tc.tile_pool(name="work", bufs=3) as wp,
        tc.tile_pool(name="psacc", bufs=1, space="PSUM") as psp,
        tc.tile_pool(name="pswork", bufs=2, space="PSUM") as psw,
    ):
        xball = pp.tile([P, NT * D], BF16)    # bf16 x, tile i at [:, i*D:]
        g_all = pp.tile([P, NT * D], BF16)    # f * x
        xbT = pp.tile([P, 2 * NL], BF16)      # local x.T; chunk h at h*NL+i*P
        ss = pp.tile([P, NT], F32)            # ||x||^2
        rss = pp.tile([P, NT], F32)           # 1/||x||^2
        invnb = pp.tile([P, NT], BF16)        # 1/||x||
        u_all = pp.tile([P, NT], F32)         # x @ t
        s_all = pp.tile([P, NT], F32)         # degrees
        rec = pp.tile([P, NT], F32)           # 1/s
        d_all = pp.tile([P, NT], F32)         # rsqrt(s)
        f_all = pp.tile([P, NT], F32)         # d / ||x||
        t16 = pp.tile([1, D], BF16)
        tb16 = pp.tile([P, D], BF16)
        W_sb = pp.tile([P, 2 * D], F32)
        Wb = pp.tile([P, 2 * D], BF16)
        zb = pp.tile([P, 2 * D], BF16)
        zwb = pp.tile([P, 2 * D], BF16)
        ones1 = pp.tile([1, P], BF16)
        identb = pp.tile([P, P], BF16)

        nc.gpsimd.memset(ones1[:], 1.0)
        masks.make_identity(nc, identb[:])

        psum_t = psp.tile([1, D], F32, padded_shape=[1, 512])
        pz0 = psp.tile([P, D], F32, padded_shape=[P, 512])
        pz1 = psp.tile([P, D], F32, padded_shape=[P, 512])

        # ---- phase A: stream x; cast, row norms, colsum t, local x.T ----
        for c in range(NCH):
            cs = slice(c * CH, (c + 1) * CH)
            xs = wp.tile([P, CH * D], F32, tag="xs", name=f"xs{c}")
            src = _AP(x.tensor, x.offset + c * CH * P * D,
                      [[D, P], [P * D, CH], [1, D]])
            nc.sync.dma_start(xs[:].rearrange("p (j d) -> p j d", j=CH), src)
            xbc = xball[:, c * CH * D:(c + 1) * CH * D]
            if c in POOL_CAST_CHUNKS:
                nc.gpsimd.tensor_copy(xbc, xs[:])
            else:
                nc.scalar.copy(xbc, xs[:])
            if c in ACT_SS_CHUNKS:
                # row sum-squares via ACT Square+accumulate (from fp32 xs)
                for j in range(CH):
                    i = c * CH + j
                    sq = wp.tile([P, D], BF16, tag="sq", name=f"sq{i}")
                    nc.scalar.activation(sq[:], xs[:, j * D:(j + 1) * D],
                                         AF.Square, accum_out=ss[:, i:i + 1])
            else:
                scrq = wp.tile([P, CH * D], BF16, tag="scrq", name=f"scrq{c}")
                nc.vector.tensor_mul(
                    scrq[:].rearrange("p (j d) -> p j d", j=CH),
                    xbc.rearrange("p (j d) -> p j d", j=CH),
                    xbc.rearrange("p (j d) -> p j d", j=CH))
                with nc.allow_low_precision("row-norm accumulate"):
                    nc.vector.tensor_reduce(
                        ss[:, cs], scrq[:].rearrange("p (j d) -> p j d", j=CH),
                        axis=mybir.AxisListType.X, op=ALU.add)
            nc.vector.reciprocal(rss[:, cs], ss[:, cs])
            nc.scalar.activation(invnb[:, cs], rss[:, cs], AF.Sqrt)
            for j in range(CH):
                i = c * CH + j
                nc.tensor.matmul(psum_t[:], lhsT=invnb[:, i:i + 1],
                                 rhs=xball[:, i * D:(i + 1) * D],
                                 start=(i == 0), stop=(i == NT - 1))
            if c == 0:  # local tiles: build x.T via PE transposes
                for j in range(CH):
                    i = j
                    for h in range(2):
                        pt = psw.tile([P, P], BF16, tag="pt", name=f"pt{i}_{h}")
                        nc.tensor.transpose(
                            pt[:], xball[:, i * D + h * P: i * D + (h + 1) * P],
                            identb[:])
                        if (i + h) % 2 == 0:
                            nc.vector.tensor_copy(
                                xbT[:, h * NL + i * P: h * NL + (i + 1) * P], pt[:])
                        else:
                            nc.scalar.copy(
                                xbT[:, h * NL + i * P: h * NL + (i + 1) * P], pt[:])

        # W load on the scalar-engine DMA queue (off the x stream)
        for kc in range(2):
            nc.scalar.dma_start(W_sb[:, kc * D:(kc + 1) * D],
                                W[kc * P:(kc + 1) * P, :])
        nc.vector.tensor_copy(Wb[:], W_sb[:])

        # ---- t -> broadcast to all partitions ----
        nc.vector.tensor_copy(t16[:], psum_t[:])
        ptb = psw.tile([P, D], F32, tag="pw", name="ptb")
        nc.tensor.matmul(ptb[:], lhsT=ones1[:], rhs=t16[:], start=True, stop=True)
        nc.vector.tensor_copy(tb16[:], ptb[:])

        # ---- phase B: degrees, f, g = f*x, dense z accumulation ----
        for gi in range(NG):
            gs = slice(gi * GRP, (gi + 1) * GRP)
            scrs = wp.tile([P, GRP * D], BF16, tag="scrs", name=f"scrs{gi}")
            if gi in POOL_SMUL_GROUPS:
                for j in range(GRP):
                    i = gi * GRP + j
                    nc.gpsimd.tensor_tensor(
                        scrs[:, j * D:(j + 1) * D],
                        xball[:, i * D:(i + 1) * D], tb16[:], op=ALU.mult)
            else:
                nc.vector.tensor_mul(
                    scrs[:].rearrange("p (t d) -> p t d", t=GRP),
                    xball[:, gi * GRP * D:(gi + 1) * GRP * D].rearrange(
                        "p (t d) -> p t d", t=GRP),
                    _rep(tb16[:], GRP, 1))
            if gi in DVE_SRED_GROUPS:
                with nc.allow_low_precision("degree accumulate"):
                    nc.vector.tensor_reduce(
                        u_all[:, gs],
                        scrs[:].rearrange("p (t d) -> p t d", t=GRP),
                        axis=mybir.AxisListType.X, op=ALU.add)
            else:
                for j in range(GRP):
                    i = gi * GRP + j
                    sq = wp.tile([P, D], BF16, tag="sq", name=f"sr{i}")
                    nc.scalar.activation(sq[:], scrs[:, j * D:(j + 1) * D],
                                         AF.Copy, accum_out=u_all[:, i:i + 1])
            # f = d * invn = sqrt(invn / u): two tiny DVE ops + one tiny ACT op
            nc.vector.reciprocal(rec[:, gs], u_all[:, gs])
            nc.vector.tensor_mul(s_all[:, gs], rec[:, gs], invnb[:, gs])
            nc.scalar.activation(f_all[:, gs], s_all[:, gs], AF.Sqrt)
            if gi in ACT_G_GROUPS:
                for j in range(GRP):
                    i = gi * GRP + j
                    nc.scalar.mul(g_all[:, i * D:(i + 1) * D],
                                  xball[:, i * D:(i + 1) * D], f_all[:, i:i + 1])
            else:
                nc.vector.tensor_mul(
                    g_all[:, gi * GRP * D:(gi + 1) * GRP * D].rearrange(
                        "p (t d) -> p t d", t=GRP),
                    xball[:, gi * GRP * D:(gi + 1) * GRP * D].rearrange(
                        "p (t d) -> p t d", t=GRP),
                    _rep(f_all[:, gs], D, 2))
            for j in range(GRP):
                i = gi * GRP + j
                for h, pz in ((0, pz0), (1, pz1)):
                    nc.tensor.matmul(
                        pz[:],
                        lhsT=g_all[:, i * D + h * P: i * D + (h + 1) * P],
                        rhs=xball[:, i * D:(i + 1) * D],
                        start=(i == 0), stop=(i == NT - 1))

        nc.vector.tensor_copy(zb[:, 0:D], pz0[:])
        nc.vector.tensor_copy(zb[:, D:2 * D], pz1[:])

        # ---- zw = z @ W ----
        for m in range(2):
            pzw = psw.tile([P, D], F32, tag="pw", name=f"pzw{m}")
            for h in range(2):
                nc.tensor.matmul(pzw[:],
                                 lhsT=zb[:, h * D + m * P: h * D + (m + 1) * P],
                                 rhs=Wb[:, h * D:(h + 1) * D],
                                 start=(h == 0), stop=(h == 1))
            nc.vector.tensor_copy(zwb[:, m * D:(m + 1) * D], pzw[:])

        # ---- phase C: out = diag(f) x_local (zw) ----
        for i in range(LT):
            po = psw.tile([P, D], F32, tag="pw", name=f"po{i}")
            for h in range(2):
                nc.tensor.matmul(po[:],
                                 lhsT=xbT[:, h * NL + i * P: h * NL + (i + 1) * P],
                                 rhs=zwb[:, h * D:(h + 1) * D],
                                 start=(h == 0), stop=(h == 1))
            o_sb = wp.tile([P, D], F32, tag="o", name=f"o{i}")
            nc.vector.tensor_scalar_mul(o_sb[:], po[:], f_all[:, i:i + 1])
            nc.sync.dma_start(out[i * P:(i + 1) * P, :], o_sb[:])


def _build():
    nc = bacc.Bacc("TRN2", target_bir_lowering=False, debug=False, num_devices=R)
    x = nc.dram_tensor("x", [N, D], F32, kind="ExternalInput")
    W = nc.dram_tensor("W", [D, D], F32, kind="ExternalInput")
    out = nc.dram_tensor("out", [NL, D], F32, kind="ExternalOutput")
    with tile.TileContext(nc) as tc:
        _program(tc, x.ap() if hasattr(x, "ap") else x,
                 W.ap() if hasattr(W, "ap") else W,
                 out.ap() if hasattr(out, "ap") else out)
    nc.finalize()
    return nc


def _run(inputs, trace=False):
    if "nc" not in _cache:
        _cache["nc"] = _build()
    nc = _cache["nc"]
    x = np.ascontiguousarray(inputs["x"], dtype=np.float32)
    W = np.ascontiguousarray(inputs["W"], dtype=np.float32)
    in_maps = []
    for r in range(R):
        xr = np.concatenate([x[r * NL:], x[:r * NL]], axis=0) if r else x
        in_maps.append({"x": xr, "W": W})
    res = bass_utils.run_bass_kernel_spmd(
        nc, in_maps, core_ids=list(range(R)), trace=trace,
    )
    out = np.concatenate([res.results[r]["out"] for r in range(R)], axis=0)
    return out, res


def kernel(**inputs) -> np.ndarray:
    out, _ = _run(inputs, trace=False)
    return out


# revision 19
# speedup vs baseline: 1.1034x; 1.0765x over previous
"""Replicated (collective-free) Trainium2 kernel for the dense-graph GNN layer.

Math: with xn = x/||x|| (rows), G = xn@xn.T, d = rsqrt(G@1),
out = (diag(d) G diag(d) x) W.  The N x N Gram matrix is never formed, and
the normalized matrix xn is never materialized (norms cancel):
  t = colsum(xn)  (PE: invn-weighted colsum of raw x)
  u = x @ t,  s = u * invn,  d = rsqrt(s),  f = d * invn
  z = x.T @ diag(f) @ x  (symmetric);  out = diag(f) @ x @ (z @ W)

Distribution: NO collectives.  Each core streams the FULL x from HBM
(DMA-overlapped) and redundantly computes the tiny globals; core r gets a
row-rotated x so a rank-agnostic program emits rows r*1024..(r+1)*1024.

Engine split (measured rates): ACT casts most of the stream + some row
reductions via activation accumulate; DVE does batched bf16 multiplies
(2x mode) + batched reduces + per-tile f-scaling; Pool casts some chunks
and does some per-tile s-dot multiplies; PE runs the colsum during the
stream and the dense z accumulation in phase B.
"""

import os
import sys

import numpy as np

for _p in ("/opt/trn_rl_repo", "/root/.axon_site/_ro/trn_rl_repo"):
    if os.path.isdir(_p) and _p not in sys.path:
        sys.path.insert(0, _p)

import concourse.bacc as bacc
import concourse.mybir as mybir
import concourse.tile as tile
import concourse.masks as masks
from concourse import bass_utils
from concourse.bass_types import AP as _AP

R = 8                 # cores
N, D = 8192, 256
NL = N // R           # 1024 output rows per core
P = 128
NT = N // P           # 64 row tiles streamed per core
CH = 8                # tiles per input DMA chunk
NCH = NT // CH        # 8 chunks
LT = NL // P          # 8 local (output) tiles = tiles 0..7 of the rotated view
GRP = 8               # tiles per s/g/z pipeline group
NG = NT // GRP
F32 = mybir.dt.float32
BF16 = mybir.dt.bfloat16
AF = mybir.ActivationFunctionType
ALU = mybir.AluOpType

POOL_CAST_CHUNKS = set()        # pool casts measured too slow (3.5 ns/elem)
ACT_SS_CHUNKS = {0}             # chunks whose row-norm accumulate runs on ACT
DVE_SRED_GROUPS = {0, 2, 3, 5, 6, 7}  # s-reduce on DVE (rest ACT)
POOL_SMUL_GROUPS = {1, 4, 7}    # groups whose s-multiply runs on Pool
ACT_G_GROUPS = {2, 5, 6}        # groups whose f-scale runs per-tile on ACT

_cache = {}


def _rep(ap, n, pos):
    """Insert a stride-0 broadcast dim of size n at free position pos."""
    dims = list(ap.ap)
    dims.insert(pos, [0, n])
    return _AP(ap.tensor, ap.offset, dims)


def _program(tc, x, W, out):
    nc = tc.nc
    with (
        tc.tile_pool(name="persist", bufs=1) as pp,
        tc.tile_pool(name="work", bufs=3) as wp,
        tc.tile_pool(name="psacc", bufs=1, space="PSUM") as psp,
        tc.tile_pool(name="pswork", bufs=2, space="PSUM") as psw,
    ):
        xball = pp.tile([P, NT * D], BF16)    # bf16 x, tile i at [:, i*D:]
        g_all = pp.tile([P, NT * D], BF16)    # f * x
        xbT = pp.tile([P, 2 * NL], BF16)      # local x.T; chunk h at h*NL+i*P
        ss = pp.tile([P, NT], F32)            # ||x||^2
        rss = pp.tile([P, NT], F32)           # 1/||x||^2
        invnb = pp.tile([P, NT], BF16)        # 1/||x||
        u_all = pp.tile([P, NT], F32)         # x @ t
        s_all = pp.tile([P, NT], F32)         # degrees
        rec = pp.tile([P, NT], F32)           # 1/s
        d_all = pp.tile([P, NT], F32)         # rsqrt(s)
        f_all = pp.tile([P, NT], F32)         # d / ||x||
        t16 = pp.tile([1, D], BF16)
        tb16 = pp.tile([P, D], BF16)
        W_sb = pp.tile([P, 2 * D], F32)
        Wb = pp.tile([P, 2 * D], BF16)
        zb = pp.tile([P, 2 * D], BF16)
        zwb = pp.tile([P, 2 * D], BF16)
        ones1 = pp.tile([1, P], BF16)
        identb = pp.tile([P, P], BF16)

        nc.gpsimd.memset(ones1[:], 1.0)
        masks.make_identity(nc, identb[:])

        psum_t = psp.tile([1, D], F32, padded_shape=[1, 512])
        pz0 = psp.tile([P, D], F32, padded_shape=[P, 512])
        pz1 = psp.tile([P, D], F32, padded_shape=[P, 512])

        # ---- phase A: stream x; cast, row norms, colsum t, local x.T ----
        for c in range(NCH):
            cs = slice(c * CH, (c + 1) * CH)
            xs = wp.tile([P, CH * D], F32, tag="xs", name=f"xs{c}")
            src = _AP(x.tensor, x.offset + c * CH * P * D,
                      [[D, P], [P * D, CH], [1, D]])
            nc.sync.dma_start(xs[:].rearrange("p (j d) -> p j d", j=CH), src)
            xbc = xball[:, c * CH * D:(c + 1) * CH * D]
            if c in POOL_CAST_CHUNKS:
                nc.gpsimd.tensor_copy(xbc, xs[:])
            else:
                nc.scalar.copy(xbc, xs[:])
            if c in ACT_SS_CHUNKS:
                # row sum-squares via ACT Square+accumulate (from fp32 xs)
                for j in range(CH):
                    i = c * CH + j
                    sq = wp.tile([P, D], BF16, tag="sq", name=f"sq{i}")
                    nc.scalar.activation(sq[:], xs[:, j * D:(j + 1) * D],
                                         AF.Square, accum_out=ss[:, i:i + 1])
            else:
                scrq = wp.tile([P, CH * D], BF16, tag="scrq", name=f"scrq{c}")
                nc.vector.tensor_mul(
                    scrq[:].rearrange("p (j d) -> p j d", j=CH),
                    xbc.rearrange("p (j d) -> p j d", j=CH),
                    xbc.rearrange("p (j d) -> p j d", j=CH))
                with nc.allow_low_precision("row-norm accumulate"):
                    nc.vector.tensor_reduce(
                        ss[:, cs], scrq[:].rearrange("p (j d) -> p j d", j=CH),
                        axis=mybir.AxisListType.X, op=ALU.add)
            nc.vector.reciprocal(rss[:, cs], ss[:, cs])
            nc.scalar.activation(invnb[:, cs], rss[:, cs], AF.Sqrt)
            for j in range(CH):
                i = c * CH + j
                nc.tensor.matmul(psum_t[:], lhsT=invnb[:, i:i + 1],
                                 rhs=xball[:, i * D:(i + 1) * D],
                                 start=(i == 0), stop=(i == NT - 1))
            if c == 0:  # local tiles: build x.T via PE transposes
                for j in range(CH):
                    i = j
                    for h in range(2):
                        pt = psw.tile([P, P], BF16, tag="pt", name=f"pt{i}_{h}")
                        nc.tensor.transpose(
                            pt[:], xball[:, i * D + h * P: i * D + (h + 1) * P],
                            identb[:])
                        nc.vector.tensor_copy(
                            xbT[:, h * NL + i * P: h * NL + (i + 1) * P], pt[:])

        # W load on the scalar-engine DMA queue (off the x stream)
        for kc in range(2):
            nc.scalar.dma_start(W_sb[:, kc * D:(kc + 1) * D],
                                W[kc * P:(kc + 1) * P, :])
        nc.vector.tensor_copy(Wb[:], W_sb[:])

        # ---- t -> broadcast to all partitions ----
        nc.vector.tensor_copy(t16[:], psum_t[:])
        ptb = psw.tile([P, D], F32, tag="pw", name="ptb")
        nc.tensor.matmul(ptb[:], lhsT=ones1[:], rhs=t16[:], start=True, stop=True)
        nc.vector.tensor_copy(tb16[:], ptb[:])

        # ---- phase B: degrees/f chains, with g+z software-pipelined one
        # group behind so the PE z train never waits on the current chain ----
        def _chain(gi):
            gs = slice(gi * GRP, (gi + 1) * GRP)
            scrs = wp.tile([P, GRP * D], BF16, tag="scrs", name=f"scrs{gi}")
            if gi in POOL_SMUL_GROUPS:
                for j in range(GRP):
                    i = gi * GRP + j
                    nc.gpsimd.tensor_tensor(
                        scrs[:, j * D:(j + 1) * D],
                        xball[:, i * D:(i + 1) * D], tb16[:], op=ALU.mult)
            else:
                nc.vector.tensor_mul(
                    scrs[:].rearrange("p (t d) -> p t d", t=GRP),
                    xball[:, gi * GRP * D:(gi + 1) * GRP * D].rearrange(
                        "p (t d) -> p t d", t=GRP),
                    _rep(tb16[:], GRP, 1))
            if gi in DVE_SRED_GROUPS:
                with nc.allow_low_precision("degree accumulate"):
                    nc.vector.tensor_reduce(
                        u_all[:, gs],
                        scrs[:].rearrange("p (t d) -> p t d", t=GRP),
                        axis=mybir.AxisListType.X, op=ALU.add)
            else:
                for j in range(GRP):
                    i = gi * GRP + j
                    sq = wp.tile([P, D], BF16, tag="sq", name=f"sr{i}")
                    nc.scalar.activation(sq[:], scrs[:, j * D:(j + 1) * D],
                                         AF.Copy, accum_out=u_all[:, i:i + 1])
            # f = d * invn = sqrt(invn / u)
            nc.vector.reciprocal(rec[:, gs], u_all[:, gs])
            nc.vector.tensor_mul(s_all[:, gs], rec[:, gs], invnb[:, gs])
            nc.scalar.activation(f_all[:, gs], s_all[:, gs], AF.Sqrt)

        def _gz(gi):
            gs = slice(gi * GRP, (gi + 1) * GRP)
            if gi in ACT_G_GROUPS:
                for j in range(GRP):
                    i = gi * GRP + j
                    nc.scalar.mul(g_all[:, i * D:(i + 1) * D],
                                  xball[:, i * D:(i + 1) * D], f_all[:, i:i + 1])
            else:
                nc.vector.tensor_mul(
                    g_all[:, gi * GRP * D:(gi + 1) * GRP * D].rearrange(
                        "p (t d) -> p t d", t=GRP),
                    xball[:, gi * GRP * D:(gi + 1) * GRP * D].rearrange(
                        "p (t d) -> p t d", t=GRP),
                    _rep(f_all[:, gs], D, 2))
            for j in range(GRP):
                i = gi * GRP + j
                for h, pz in ((0, pz0), (1, pz1)):
                    nc.tensor.matmul(
                        pz[:],
                        lhsT=g_all[:, i * D + h * P: i * D + (h + 1) * P],
                        rhs=xball[:, i * D:(i + 1) * D],
                        start=(i == 0), stop=(i == NT - 1))

        for gi in range(NG):
            _chain(gi)
            if gi >= 1:
                _gz(gi - 1)
        _gz(NG - 1)

        nc.vector.tensor_copy(zb[:, 0:D], pz0[:])
        nc.vector.tensor_copy(zb[:, D:2 * D], pz1[:])

        # ---- zw = z @ W ----
        for m in range(2):
            pzw = psw.tile([P, D], F32, tag="pw", name=f"pzw{m}")
            for h in range(2):
                nc.tensor.matmul(pzw[:],
                                 lhsT=zb[:, h * D + m * P: h * D + (m + 1) * P],
                                 rhs=Wb[:, h * D:(h + 1) * D],
                                 start=(h == 0), stop=(h == 1))
            nc.vector.tensor_copy(zwb[:, m * D:(m + 1) * D], pzw[:])

        # ---- phase C: out = diag(f) x_local (zw) ----
        for i in range(LT):
            po = psw.tile([P, D], F32, tag="pw", name=f"po{i}")
            for h in range(2):
                nc.tensor.matmul(po[:],
                                 lhsT=xbT[:, h * NL + i * P: h * NL + (i + 1) * P],
                                 rhs=zwb[:, h * D:(h + 1) * D],
                                 start=(h == 0), stop=(h == 1))
            o_sb = wp.tile([P, D], F32, tag="o", name=f"o{i}")
            nc.vector.tensor_scalar_mul(o_sb[:], po[:], f_all[:, i:i + 1])
            nc.sync.dma_start(out[i * P:(i + 1) * P, :], o_sb[:])


def _build():
    nc = bacc.Bacc("TRN2", target_bir_lowering=False, debug=False, num_devices=R)
    x = nc.dram_tensor("x", [N, D], F32, kind="ExternalInput")
    W = nc.dram_tensor("W", [D, D], F32, kind="ExternalInput")
    out = nc.dram_tensor("out", [NL, D], F32, kind="ExternalOutput")
    with tile.TileContext(nc) as tc:
        _program(tc, x.ap() if hasattr(x, "ap") else x,
                 W.ap() if hasattr(W, "ap") else W,
                 out.ap() if hasattr(out, "ap") else out)
    nc.finalize()
    return nc


def _run(inputs, trace=False):
    if "nc" not in _cache:
        _cache["nc"] = _build()
    nc = _cache["nc"]
    x = np.ascontiguousarray(inputs["x"], dtype=np.float32)
    W = np.ascontiguousarray(inputs["W"], dtype=np.float32)
    in_maps = []
    for r in range(R):
        xr = np.concatenate([x[r * NL:], x[:r * NL]], axis=0) if r else x
        in_maps.append({"x": xr, "W": W})
    res = bass_utils.run_bass_kernel_spmd(
        nc, in_maps, core_ids=list(range(R)), trace=trace,
    )
    out = np.concatenate([res.results[r]["out"] for r in range(R)], axis=0)
    return out, res


def kernel(**inputs) -> np.ndarray:
    out, _ = _run(inputs, trace=False)
    return out


# revision 20
# speedup vs baseline: 1.1035x; 1.0000x over previous
"""Replicated (collective-free) Trainium2 kernel for the dense-graph GNN layer.

Math: with xn = x/||x|| (rows), G = xn@xn.T, d = rsqrt(G@1),
out = (diag(d) G diag(d) x) W.  The N x N Gram matrix is never formed, and
the normalized matrix xn is never materialized (norms cancel):
  t = colsum(xn)  (PE: invn-weighted colsum of raw x)
  u = x @ t,  s = u * invn,  d = rsqrt(s),  f = d * invn
  z = x.T @ diag(f) @ x  (symmetric);  out = diag(f) @ x @ (z @ W)

Distribution: NO collectives.  Each core streams the FULL x from HBM
(DMA-overlapped) and redundantly computes the tiny globals; core r gets a
row-rotated x so a rank-agnostic program emits rows r*1024..(r+1)*1024.

Engine split (measured rates): ACT casts most of the stream + some row
reductions via activation accumulate; DVE does batched bf16 multiplies
(2x mode) + batched reduces + per-tile f-scaling; Pool casts some chunks
and does some per-tile s-dot multiplies; PE runs the colsum during the
stream and the dense z accumulation in phase B.
"""

import os
import sys

import numpy as np

for _p in ("/opt/trn_rl_repo", "/root/.axon_site/_ro/trn_rl_repo"):
    if os.path.isdir(_p) and _p not in sys.path:
        sys.path.insert(0, _p)

import concourse.bacc as bacc
import concourse.mybir as mybir
import concourse.tile as tile
import concourse.masks as masks
from concourse import bass_utils
from concourse.bass_types import AP as _AP

R = 8                 # cores
N, D = 8192, 256
NL = N // R           # 1024 output rows per core
P = 128
NT = N // P           # 64 row tiles streamed per core
CH = 8                # tiles per input DMA chunk
NCH = NT // CH        # 8 chunks
LT = NL // P          # 8 local (output) tiles = tiles 0..7 of the rotated view
GRP = 8               # tiles per s/g/z pipeline group
NG = NT // GRP
F32 = mybir.dt.float32
BF16 = mybir.dt.bfloat16
AF = mybir.ActivationFunctionType
ALU = mybir.AluOpType

POOL_CAST_CHUNKS = set()        # pool casts measured too slow (3.5 ns/elem)
ACT_SS_CHUNKS = {0}             # chunks whose row-norm accumulate runs on ACT
DVE_SRED_GROUPS = {0, 2, 3, 5, 6, 7}  # s-reduce on DVE (rest ACT)
POOL_SMUL_GROUPS = {1, 4, 7}    # groups whose s-multiply runs on Pool
ACT_G_GROUPS = {2, 5, 6}        # groups whose f-scale runs per-tile on ACT

_cache = {}


def _rep(ap, n, pos):
    """Insert a stride-0 broadcast dim of size n at free position pos."""
    dims = list(ap.ap)
    dims.insert(pos, [0, n])
    return _AP(ap.tensor, ap.offset, dims)


def _program(tc, x, W, out):
    nc = tc.nc
    with (
        tc.tile_pool(name="persist", bufs=1) as pp,
        tc.tile_pool(name="work", bufs=3) as wp,
        tc.tile_pool(name="psacc", bufs=1, space="PSUM") as psp,
        tc.tile_pool(name="pswork", bufs=2, space="PSUM") as psw,
    ):
        xball = pp.tile([P, NT * D], BF16)    # bf16 x, tile i at [:, i*D:]
        g_all = pp.tile([P, NT * D], BF16)    # f * x
        xbT = pp.tile([P, 2 * NL], BF16)      # local x.T; chunk h at h*NL+i*P
        ss = pp.tile([P, NT], F32)            # ||x||^2
        rss = pp.tile([P, NT], F32)           # 1/||x||^2
        invnb = pp.tile([P, NT], BF16)        # 1/||x||
        u_all = pp.tile([P, NT], F32)         # x @ t
        s_all = pp.tile([P, NT], F32)         # degrees
        rec = pp.tile([P, NT], F32)           # 1/s
        d_all = pp.tile([P, NT], F32)         # rsqrt(s)
        f_all = pp.tile([P, NT], F32)         # d / ||x||
        t16 = pp.tile([1, D], BF16)
        tb16 = pp.tile([P, D], BF16)
        W_sb = pp.tile([P, 2 * D], F32)
        Wb = pp.tile([P, 2 * D], BF16)
        zb = pp.tile([P, 2 * D], BF16)
        zwb = pp.tile([P, 2 * D], BF16)
        ones1 = pp.tile([1, P], BF16)
        identb = pp.tile([P, P], BF16)

        nc.gpsimd.memset(ones1[:], 1.0)
        masks.make_identity(nc, identb[:])

        psum_t = psp.tile([1, D], F32, padded_shape=[1, 512])
        pz0 = psp.tile([P, D], F32, padded_shape=[P, 512])
        pz1 = psp.tile([P, D], F32, padded_shape=[P, 512])

        # ---- phase A: stream x; cast, row norms, colsum t, local x.T ----
        for c in range(NCH):
            cs = slice(c * CH, (c + 1) * CH)
            xs = wp.tile([P, CH * D], F32, tag="xs", name=f"xs{c}")
            src = _AP(x.tensor, x.offset + c * CH * P * D,
                      [[D, P], [P * D, CH], [1, D]])
            nc.sync.dma_start(xs[:].rearrange("p (j d) -> p j d", j=CH), src)
            xbc = xball[:, c * CH * D:(c + 1) * CH * D]
            if c in POOL_CAST_CHUNKS:
                nc.gpsimd.tensor_copy(xbc, xs[:])
            else:
                nc.scalar.copy(xbc, xs[:])
            if c in ACT_SS_CHUNKS:
                # row sum-squares via ACT Square+accumulate (from fp32 xs)
                for j in range(CH):
                    i = c * CH + j
                    sq = wp.tile([P, D], BF16, tag="sq", name=f"sq{i}")
                    nc.scalar.activation(sq[:], xs[:, j * D:(j + 1) * D],
                                         AF.Square, accum_out=ss[:, i:i + 1])
            elif c == 1:
                # split: first half of the chunk on ACT, second half on DVE
                for j in range(CH // 2):
                    i = c * CH + j
                    sq = wp.tile([P, D], BF16, tag="sq", name=f"sq{i}")
                    nc.scalar.activation(sq[:], xs[:, j * D:(j + 1) * D],
                                         AF.Square, accum_out=ss[:, i:i + 1])
                scrh = wp.tile([P, CH * D // 2], BF16, tag="scrq", name="scrqh")
                half = xball[:, (c * CH + CH // 2) * D:(c + 1) * CH * D]
                nc.vector.tensor_mul(
                    scrh[:].rearrange("p (j d) -> p j d", j=CH // 2),
                    half.rearrange("p (j d) -> p j d", j=CH // 2),
                    half.rearrange("p (j d) -> p j d", j=CH // 2))
                with nc.allow_low_precision("row-norm accumulate"):
                    nc.vector.tensor_reduce(
                        ss[:, c * CH + CH // 2:(c + 1) * CH],
                        scrh[:].rearrange("p (j d) -> p j d", j=CH // 2),
                        axis=mybir.AxisListType.X, op=ALU.add)
            else:
                scrq = wp.tile([P, CH * D], BF16, tag="scrq", name=f"scrq{c}")
                nc.vector.tensor_mul(
                    scrq[:].rearrange("p (j d) -> p j d", j=CH),
                    xbc.rearrange("p (j d) -> p j d", j=CH),
                    xbc.rearrange("p (j d) -> p j d", j=CH))
                with nc.allow_low_precision("row-norm accumulate"):
                    nc.vector.tensor_reduce(
                        ss[:, cs], scrq[:].rearrange("p (j d) -> p j d", j=CH),
                        axis=mybir.AxisListType.X, op=ALU.add)
            nc.vector.reciprocal(rss[:, cs], ss[:, cs])
            nc.scalar.activation(invnb[:, cs], rss[:, cs], AF.Sqrt)
            for j in range(CH):
                i = c * CH + j
                nc.tensor.matmul(psum_t[:], lhsT=invnb[:, i:i + 1],
                                 rhs=xball[:, i * D:(i + 1) * D],
                                 start=(i == 0), stop=(i == NT - 1))
            if c == 0:  # local tiles: build x.T via PE transposes
                for j in range(CH):
                    i = j
                    for h in range(2):
                        pt = psw.tile([P, P], BF16, tag="pt", name=f"pt{i}_{h}")
                        nc.tensor.transpose(
                            pt[:], xball[:, i * D + h * P: i * D + (h + 1) * P],
                            identb[:])
                        nc.vector.tensor_copy(
                            xbT[:, h * NL + i * P: h * NL + (i + 1) * P], pt[:])

        # W load on the scalar-engine DMA queue (off the x stream)
        for kc in range(2):
            nc.scalar.dma_start(W_sb[:, kc * D:(kc + 1) * D],
                                W[kc * P:(kc + 1) * P, :])
        nc.vector.tensor_copy(Wb[:], W_sb[:])

        # ---- t -> broadcast to all partitions ----
        nc.vector.tensor_copy(t16[:], psum_t[:])
        ptb = psw.tile([P, D], F32, tag="pw", name="ptb")
        nc.tensor.matmul(ptb[:], lhsT=ones1[:], rhs=t16[:], start=True, stop=True)
        nc.vector.tensor_copy(tb16[:], ptb[:])

        # ---- phase B: degrees/f chains, with g+z software-pipelined one
        # group behind so the PE z train never waits on the current chain ----
        def _chain(gi):
            gs = slice(gi * GRP, (gi + 1) * GRP)
            scrs = wp.tile([P, GRP * D], BF16, tag="scrs", name=f"scrs{gi}")
            if gi in POOL_SMUL_GROUPS:
                for j in range(GRP):
                    i = gi * GRP + j
                    nc.gpsimd.tensor_tensor(
                        scrs[:, j * D:(j + 1) * D],
                        xball[:, i * D:(i + 1) * D], tb16[:], op=ALU.mult)
            else:
                nc.vector.tensor_mul(
                    scrs[:].rearrange("p (t d) -> p t d", t=GRP),
                    xball[:, gi * GRP * D:(gi + 1) * GRP * D].rearrange(
                        "p (t d) -> p t d", t=GRP),
                    _rep(tb16[:], GRP, 1))
            if gi in DVE_SRED_GROUPS:
                with nc.allow_low_precision("degree accumulate"):
                    nc.vector.tensor_reduce(
                        u_all[:, gs],
                        scrs[:].rearrange("p (t d) -> p t d", t=GRP),
                        axis=mybir.AxisListType.X, op=ALU.add)
            else:
                for j in range(GRP):
                    i = gi * GRP + j
                    sq = wp.tile([P, D], BF16, tag="sq", name=f"sr{i}")
                    nc.scalar.activation(sq[:], scrs[:, j * D:(j + 1) * D],
                                         AF.Copy, accum_out=u_all[:, i:i + 1])
            # f = d * invn = sqrt(invn / u)
            nc.vector.reciprocal(rec[:, gs], u_all[:, gs])
            nc.vector.tensor_mul(s_all[:, gs], rec[:, gs], invnb[:, gs])
            nc.scalar.activation(f_all[:, gs], s_all[:, gs], AF.Sqrt)

        def _gz(gi):
            gs = slice(gi * GRP, (gi + 1) * GRP)
            if gi in ACT_G_GROUPS:
                for j in range(GRP):
                    i = gi * GRP + j
                    nc.scalar.mul(g_all[:, i * D:(i + 1) * D],
                                  xball[:, i * D:(i + 1) * D], f_all[:, i:i + 1])
            else:
                nc.vector.tensor_mul(
                    g_all[:, gi * GRP * D:(gi + 1) * GRP * D].rearrange(
                        "p (t d) -> p t d", t=GRP),
                    xball[:, gi * GRP * D:(gi + 1) * GRP * D].rearrange(
                        "p (t d) -> p t d", t=GRP),
                    _rep(f_all[:, gs], D, 2))
            for j in range(GRP):
                i = gi * GRP + j
                for h, pz in ((0, pz0), (1, pz1)):
                    nc.tensor.matmul(
                        pz[:],
                        lhsT=g_all[:, i * D + h * P: i * D + (h + 1) * P],
                        rhs=xball[:, i * D:(i + 1) * D],
                        start=(i == 0), stop=(i == NT - 1))

        for gi in range(NG):
            _chain(gi)
            if gi >= 1:
                _gz(gi - 1)
        _gz(NG - 1)

        nc.vector.tensor_copy(zb[:, 0:D], pz0[:])
        nc.vector.tensor_copy(zb[:, D:2 * D], pz1[:])

        # ---- zw = z @ W ----
        for m in range(2):
            pzw = psw.tile([P, D], F32, tag="pw", name=f"pzw{m}")
            for h in range(2):
                nc.tensor.matmul(pzw[:],
                                 lhsT=zb[:, h * D + m * P: h * D + (m + 1) * P],
                                 rhs=Wb[:, h * D:(h + 1) * D],
                                 start=(h == 0), stop=(h == 1))
            nc.vector.tensor_copy(zwb[:, m * D:(m + 1) * D], pzw[:])

        # ---- phase C: out = diag(f) x_local (zw) ----
        for i in range(LT):
            po = psw.tile([P, D], F32, tag="pw", name=f"po{i}")
            for h in range(2):
                nc.tensor.matmul(po[:],
                                 lhsT=xbT[:, h * NL + i * P: h * NL + (i + 1) * P],
                                 rhs=zwb[:, h * D:(h + 1) * D],
                                 start=(h == 0), stop=(h == 1))
            o_sb = wp.tile([P, D], F32, tag="o", name=f"o{i}")
            nc.vector.tensor_scalar_mul(o_sb[:], po[:], f_all[:, i:i + 1])
            nc.sync.dma_start(out[i * P:(i + 1) * P, :], o_sb[:])


def _build():
    nc = bacc.Bacc("TRN2", target_bir_lowering=False, debug=False, num_devices=R)
    x = nc.dram_tensor("x", [N, D], F32, kind="ExternalInput")
    W = nc.dram_tensor("W", [D, D], F32, kind="ExternalInput")
    out = nc.dram_tensor("out", [NL, D], F32, kind="ExternalOutput")
    with tile.TileContext(nc) as tc:
        _program(tc, x.ap() if hasattr(x, "ap") else x,
                 W.ap() if hasattr(W, "ap") else W,
                 out.ap() if hasattr(out, "ap") else out)
    nc.finalize()
    return nc


def _run(inputs, trace=False):
    if "nc" not in _cache:
        _cache["nc"] = _build()
    nc = _cache["nc"]
    x = np.ascontiguousarray(inputs["x"], dtype=np.float32)
    W = np.ascontiguousarray(inputs["W"], dtype=np.float32)
    in_maps = []
    for r in range(R):
        xr = np.concatenate([x[r * NL:], x[:r * NL]], axis=0) if r else x
        in_maps.append({"x": xr, "W": W})
    res = bass_utils.run_bass_kernel_spmd(
        nc, in_maps, core_ids=list(range(R)), trace=trace,
    )
    out = np.concatenate([res.results[r]["out"] for r in range(R)], axis=0)
    return out, res


def kernel(**inputs) -> np.ndarray:
    out, _ = _run(inputs, trace=False)
    return out
